# revision 42
# baseline (speedup 1.0000x reference)
"""Trainium2 Bass kernel for the MGA dense-transformer block (v3).

Reference computation (per batch n):
    qkv = depthwise3(conv1x1(x, w_qkv), w_dw)         # (3D, L)
    q,k,v per head (dh=64), l2-normalized q,k, scores = q k^T * temp,
    softmax over keys, out = attn @ v, y = conv1x1(out, w_proj)

Sharding over 8 cores: core c -> (batch n = c//2, head group g = c%2 of 4
heads).  Each core computes its 768 qkv channels, runs attention for its 4
heads, and produces a partial projection y_partial = Wp[:, cols_g] @ out_g
(512, 2048).  Host sums the two partials per batch.

Engine balance (v3):
  * everything bf16 except PSUM accumulation and the norm/denominator
    chains: halves DMA + SBUF and doubles DVE throughput on the
    depthwise taps; matmuls are 1 cycle/row either way.
  * depthwise: center tap on Pool, side taps on DVE (2x bf16 mode),
    PSUM drain via Act in the prologue chunks / DVE in the steady state.
  * l2 norms: Pool squares + partition_all_reduce (result lands on all
    64 partitions), then rsqrt = exp(-0.5*ln(s)) on Act -- ln and exp
    share one activation table so the Act engine never reloads tables;
    temperature enters as an exp bias (host sends ln(temp)).
  * attention: scores f32 PSUM -> Act exp -> bf16 stripes -> PV with a
    ones row appended to v^T for the softmax denominator; 1/denom via
    DVE reciprocal + Pool partition_broadcast.
  * conv chunks for heads 2,3 are emitted between head-0 attention
    lt-blocks; projection runs per-lt right behind head-3 attention.
"""

from contextlib import ExitStack

import numpy as np

import concourse.bacc as bacc
import concourse.bass_isa as bass_isa
import concourse.mybir as mybir
import concourse.tile as tile
from concourse.bass_utils import run_bass_kernel_spmd

F32 = mybir.dt.float32
F32R = mybir.dt.float32r
BF16 = mybir.dt.bfloat16
AF = mybir.ActivationFunctionType
RED = bass_isa.ReduceOp

N, D, L, H = 4, 512, 2048, 8
DH = D // H          # 64 head dim
HPC = H // 2         # 4 heads per core
C = 3 * 256          # 768 shard qkv channels
P = 128
NLT = L // 512       # 4 query tiles
NLC = L // 128       # 16 key chunks
N_CORES = 8


def build_program(debug_dumps=False, repeat=1):
    nc = bacc.Bacc("TRN2", target_bir_lowering=False, debug=False)
    dbg = {}
    if debug_dumps:
        dbg["pre0"] = nc.dram_tensor("dbg_pre0", (P, L), BF16, kind="ExternalOutput")
        dbg["dw0"] = nc.dram_tensor("dbg_dw0", (P, L), BF16, kind="ExternalOutput")
        dbg["sq0"] = nc.dram_tensor("dbg_sq0", (P, L), BF16, kind="ExternalOutput")
        dbg["ss0"] = nc.dram_tensor("dbg_ss0", (P, L), F32, kind="ExternalOutput")
        dbg["qn0"] = nc.dram_tensor("dbg_qn0", (DH, L), BF16, kind="ExternalOutput")
        dbg["kn0"] = nc.dram_tensor("dbg_kn0", (DH, L), BF16, kind="ExternalOutput")
        dbg["vt0"] = nc.dram_tensor(
            "dbg_vt0", (P, NLC, DH + 1), BF16, kind="ExternalOutput"
        )
        dbg["es0"] = nc.dram_tensor("dbg_es0", (P, 4, 512), BF16, kind="ExternalOutput")
        dbg["outn0"] = nc.dram_tensor("dbg_outn0", (DH, L), BF16, kind="ExternalOutput")

    x_d = nc.dram_tensor("x", (D, L), BF16, kind="ExternalInput")
    wqkvT_d = nc.dram_tensor("wqkvT", (D, C), BF16, kind="ExternalInput")
    wdw_d = nc.dram_tensor("wdw", (C, 3), F32, kind="ExternalInput")
    wpT_d = nc.dram_tensor("wpT", (HPC, DH, D), BF16, kind="ExternalInput")
    # col hl: row 0 = ln(temp_hl) (q), row 32 = 0 (k); bias of the rsqrt exp
    lntemp_d = nc.dram_tensor("lntemp", (P, HPC), F32, kind="ExternalInput")
    onesc_d = nc.dram_tensor("onesc", (P, 1), BF16, kind="ExternalInput")
    # two stacked copies of eye(64) so a slice exists at base partition 0 and 64
    ident_d = nc.dram_tensor("ident", (2 * DH, DH), BF16, kind="ExternalInput")
    onesv_d = nc.dram_tensor("onesv", (P, NLC, 1), BF16, kind="ExternalInput")
    y_d = nc.dram_tensor("y", (D, L), F32, kind="ExternalOutput")

    with tile.TileContext(nc) as tc, ExitStack() as ctx:
        wp = ctx.enter_context(tc.tile_pool(name="w", bufs=1))
        xp = ctx.enter_context(tc.tile_pool(name="xp", bufs=4))
        onp = ctx.enter_context(tc.tile_pool(name="onp", bufs=8))
        esp = ctx.enter_context(tc.tile_pool(name="esp", bufs=4))
        prep = ctx.enter_context(tc.tile_pool(name="prep", bufs=2))
        dwp = ctx.enter_context(tc.tile_pool(name="dwp", bufs=6))
        nqp = ctx.enter_context(tc.tile_pool(name="nqp", bufs=4))
        vtp = ctx.enter_context(tc.tile_pool(name="vtp", bufs=1))
        smp = ctx.enter_context(tc.tile_pool(name="smp", bufs=1))
        # PSUM: pa = 1-bank slots (conv pre, po, proj), 4 bufs;
        #       pb = 2-bank slots (score pairs, transposes), 2 bufs. 4+4=8.
        pap = ctx.enter_context(tc.tile_pool(name="pa", bufs=4, space="PSUM"))
        pbp = ctx.enter_context(tc.tile_pool(name="pb", bufs=2, space="PSUM"))

        # ---- weights / constants -------------------------------------------
        wq_sb = []
        for kc in range(4):
            t = wp.tile([P, C], BF16, tag=f"wq{kc}")
            nc.gpsimd.dma_start(t[:], wqkvT_d[kc * 128:(kc + 1) * 128, :])
            wq_sb.append(t)
        wdw_sb = []
        for cc in range(6):
            t = wp.tile([P, 3], F32, tag=f"wdw{cc}")
            nc.gpsimd.dma_start(t[:], wdw_d[cc * 128:(cc + 1) * 128, :])
            wdw_sb.append(t)
        wp_sb = []
        for hl in range(HPC):
            t = wp.tile([DH, D], BF16, tag=f"wp{hl}")
            nc.gpsimd.dma_start(t[:], wpT_d[hl, :, :])
            wp_sb.append(t)
        lntemp_sb = wp.tile([P, HPC], F32, tag="lntemp")
        nc.gpsimd.dma_start(lntemp_sb[:], lntemp_d[:])
        ones_sb = wp.tile([P, 1], BF16, tag="onesc")
        nc.gpsimd.dma_start(ones_sb[:], onesc_d[:])
        ident_sb = wp.tile([2 * DH, DH], BF16, tag="ident")
        nc.gpsimd.dma_start(ident_sb[:], ident_d[:])
        onesv_sb = wp.tile([P, NLC, 1], BF16, tag="onesv")
        nc.gpsimd.dma_start(onesv_sb[:], onesv_d[:])

        x_sb = []
        dw_sb = {}
        vt_sb = {}
        outn_sb = []

        def load_x(rep):
            x_sb.clear()
            for kc in range(4):
                t = xp.tile([P, L], BF16, tag="x", name=f"x{rep}_{kc}")
                nc.sync.dma_start(t[:], x_d[kc * 128:(kc + 1) * 128, :])
                x_sb.append(t)
            outn_sb.clear()
            outn_sb.extend(
                onp.tile([DH, L], BF16, tag="outn", name=f"outn{rep}_{i}")
                for i in range(HPC)
            )

        def conv_chunk(cc, drain_act=False):
            # drain_act: early chunks drain PSUM via Act (idle in prologue);
            # later chunks drain via DVE so Act's exp stream is not stalled.
            pre = prep.tile([P, L], BF16, tag="pre", name=f"pre{cc}")
            for lt in range(NLT):
                ps = pap.tile([P, 512], F32, tag="pa", name=f"cps{cc}_{lt}")
                for kc in range(4):
                    nc.tensor.matmul(
                        ps[:],
                        wq_sb[kc][:, cc * 128:(cc + 1) * 128],
                        x_sb[kc][:, lt * 512:(lt + 1) * 512],
                        start=(kc == 0),
                        stop=(kc == 3),
                    )
                if drain_act:
                    nc.scalar.copy(pre[:, lt * 512:(lt + 1) * 512], ps[:])
                else:
                    nc.vector.tensor_copy(pre[:, lt * 512:(lt + 1) * 512], ps[:])
            dw = dwp.tile([P, L], BF16, tag="dw", name=f"dw{cc}")
            nc.gpsimd.tensor_scalar_mul(dw[:], pre[:], wdw_sb[cc][:, 1:2])
            nc.vector.affine_then_add(
                dw[:, 1:L], pre[:, 0:L - 1], dw[:, 1:L],
                scale=wdw_sb[cc][:, 0:1], bias=0.0,
            )
            nc.vector.affine_then_add(
                dw[:, 0:L - 1], pre[:, 1:L], dw[:, 0:L - 1],
                scale=wdw_sb[cc][:, 2:3], bias=0.0,
            )
            dw_sb[cc] = dw
            if debug_dumps and cc == 0:
                nc.sync.dma_start(dbg["pre0"][:], pre[:])
                nc.sync.dma_start(dbg["dw0"][:], dw[:])

        def ch_slice(base, hl):
            c0 = base + DH * hl
            return dw_sb[c0 // 128][c0 % 128:c0 % 128 + DH, :]

        def normalize(hl):
            # l2-normalize q and k of head hl in place.  Sum-of-squares via
            # a ones-vector matmul into PSUM rows {0 (q), 32 (k)}; rsqrt as
            # exp(-0.5*ln(s) + ln(temp)) on Act -- ln/exp share one
            # activation table with the attention exp, so no table reloads.
            q, k = ch_slice(0, hl), ch_slice(256, hl)
            b = DH * (hl % 2)  # base partition of this head's q/k slices
            sqq = nqp.tile([P, L], BF16, tag="nq", name=f"sqq{hl}")
            sqk = nqp.tile([P, L], BF16, tag="nq", name=f"sqk{hl}")
            nc.vector.tensor_mul(sqq[b:b + DH, :], q[:], q[:])
            nc.vector.tensor_mul(sqk[b:b + DH, :], k[:], k[:])
            nrm = smp.tile([P, L], F32, tag="nrm", bufs=2, name=f"nrm{hl}")
            for lt in range(NLT):
                ps = pap.tile([P, 512], F32, tag="pa", name=f"nps{hl}_{lt}")
                nc.tensor.matmul(
                    ps[0:1, :], ones_sb[b:b + DH, :],
                    sqq[b:b + DH, lt * 512:(lt + 1) * 512],
                    start=True, stop=True, skip_group_check=True,
                )
                nc.tensor.matmul(
                    ps[DH:DH + 1, :], ones_sb[b:b + DH, :],
                    sqk[b:b + DH, lt * 512:(lt + 1) * 512],
                    start=True, stop=True, skip_group_check=True,
                )
                nc.scalar.activation(
                    nrm[0:DH + 1, lt * 512:(lt + 1) * 512], ps[0:DH + 1, :], AF.Ln
                )
            nc.scalar.activation(
                nrm[0:DH + 1, :], nrm[0:DH + 1, :], AF.Exp, scale=-0.5,
                bias=lntemp_sb[0:DH + 1, hl:hl + 1],
            )
            # broadcast q's row, then hop k's row 64 -> row 0 within the same
            # tile (partition_broadcast replicates partition 0 of its input)
            bcq = nqp.tile([P, L], F32, tag="nq", name=f"bcq{hl}")
            bck = nqp.tile([P, L], F32, tag="nq", name=f"bck{hl}")
            nc.gpsimd.partition_broadcast(bcq[:], nrm[0:1, :])
            nc.gpsimd.tensor_copy(nrm[0:1, :], nrm[DH:DH + 1, :])
            nc.gpsimd.partition_broadcast(bck[:], nrm[0:1, :])
            if debug_dumps and hl == 0:
                nc.sync.dma_start(dbg["sq0"][:], sqq[:])
                nc.sync.dma_start(dbg["ss0"][:], bcq[:])
            nc.vector.tensor_mul(q[:], q[:], bcq[b:b + DH, :])
            nc.vector.tensor_mul(k[:], k[:], bck[b:b + DH, :])
            if debug_dumps and hl == 0:
                nc.sync.dma_start(dbg["qn0"][:], q[:])
                nc.sync.dma_start(dbg["kn0"][:], k[:])

        def build_vt(hl):
            # v^T plus a ones row for the softmax denominator, stored bf16
            v = ch_slice(512, hl)
            vt = vtp.tile([P, NLC, DH + 1], BF16, tag=f"vt{hl}", name=f"vt{hl}")
            nc.vector.tensor_copy(vt[:, :, DH:DH + 1], onesv_sb[:])
            vbase = (512 + DH * hl) % 128  # base partition of the v slice (0 or 64)
            ident = ident_sb[vbase:vbase + DH, :]
            # 4 transposes form one accumulation group in a single PSUM bank
            for lg in range(NLC // 4):
                ps = pbp.tile([P, 4, DH], BF16, tag="pb", name=f"tps{hl}_{lg}")
                for j in range(4):
                    lc = 4 * lg + j
                    nc.tensor.matmul(
                        ps[:, j, :], v[:, lc * 128:(lc + 1) * 128], ident,
                        is_transpose=True, start=(j == 0), stop=(j == 3),
                    )
                nc.vector.tensor_copy(vt[:, 4 * lg:4 * lg + 4, 0:DH], ps[:])
            vt_sb[hl] = vt
            if debug_dumps and hl == 0:
                nc.sync.dma_start(dbg["vt0"][:], vt[:])

        def attention_lt(hl, lt):
            q = ch_slice(0, hl)
            k = ch_slice(256, hl)
            vt = vt_sb[hl]
            stripes = [
                esp.tile([P, 4, 512], BF16, tag="es", name=f"es_{hl}_{lt}_{i}")
                for i in range(4)
            ]
            po = pap.tile([P, 512], F32, tag="pa", name=f"po{hl}_{lt}")
            for g in range(NLC // 2):
                ps4 = pbp.tile([P, 2, 512], F32, tag="pb", name=f"s{hl}_{lt}_{g}")
                for j in range(2):
                    lc = 2 * g + j
                    nc.tensor.matmul(
                        ps4[:, j, :],
                        k[:, lc * 128:(lc + 1) * 128],
                        q[:, lt * 512:(lt + 1) * 512],
                        start=True,
                        stop=True,
                    )
                st = stripes[g // 2]
                nc.scalar.activation(
                    st[:, 2 * (g % 2):2 * (g % 2) + 2, :], ps4[:], AF.Exp
                )
                for j in range(2):
                    lc = 2 * g + j
                    nc.tensor.matmul(
                        po[0:DH + 1, :],
                        vt[:, lc, :],
                        st[:, lc % 4, :],
                        start=(lc == 0),
                        stop=(lc == NLC - 1),
                    )
            if debug_dumps and hl == 0 and lt == 0:
                nc.sync.dma_start(dbg["es0"][:], stripes[0][:])
            rec = smp.tile([P, 512], F32, tag="rec", bufs=2)
            nc.vector.reciprocal(rec[DH:DH + 1, :], po[DH:DH + 1, :])
            # partition_broadcast replicates partition 0 of its input tile,
            # so hop the reciprocal from row 64 to row 0 on Pool first.
            nc.gpsimd.tensor_copy(rec[0:1, :], rec[DH:DH + 1, :])
            bcd = smp.tile([P, 512], F32, tag="bcd", bufs=2, name=f"bcd{hl}_{lt}")
            nc.gpsimd.partition_broadcast(bcd[0:DH, :], rec[0:1, :])
            dst = outn_sb[hl][:, lt * 512:(lt + 1) * 512]
            nc.vector.tensor_mul(dst, po[0:DH, :], bcd[0:DH, :])
            if debug_dumps and hl == 0 and lt == NLT - 1:
                nc.sync.dma_start(dbg["outn0"][:], outn_sb[0][:])

        def projection_lt(rep, lt):
            for oc in range(4):
                ps = pap.tile([P, 512], F32, tag="pa", name=f"yps{rep}_{oc}{lt}")
                for hl in range(HPC):
                    nc.tensor.matmul(
                        ps[:],
                        wp_sb[hl][:, oc * 128:(oc + 1) * 128],
                        outn_sb[hl][:, lt * 512:(lt + 1) * 512],
                        start=(hl == 0),
                        stop=(hl == HPC - 1),
                    )
                ysb = smp.tile(
                    [P, 512], F32, tag="ysb", bufs=2, name=f"ysb{rep}_{oc}_{lt}"
                )
                nc.vector.tensor_copy(ysb[:], ps[:])
                nc.sync.dma_start(
                    y_d[oc * 128:(oc + 1) * 128, lt * 512:(lt + 1) * 512], ysb[:]
                )

        for rep in range(repeat):
            load_x(rep)
            # heads 0,1 inputs first so their attention can start while the
            # remaining conv chunks run between attention lt-blocks.
            conv_chunk(0, drain_act=True)
            conv_chunk(2, drain_act=True)
            normalize(0)
            normalize(1)
            conv_chunk(4, drain_act=True)
            build_vt(0)
            build_vt(1)
            attention_lt(0, 0)
            conv_chunk(1)
            attention_lt(0, 1)
            conv_chunk(3)
            attention_lt(0, 2)
            conv_chunk(5)
            attention_lt(0, 3)
            normalize(2)
            normalize(3)
            build_vt(2)
            build_vt(3)
            for lt in range(NLT):
                attention_lt(1, lt)
            for lt in range(NLT):
                attention_lt(2, lt)
                attention_lt(3, lt)
                projection_lt(rep, lt)

    nc.compile()
    return nc


def _lntemp(temps):
    # col hl: row 0 = ln(temp_hl) for q, row 32 = 0 for k, rest unused
    t = np.zeros((P, HPC), np.float32)
    t[0, :] = np.log(temps)
    return t


def make_in_maps(x, w_qkv, w_dw, w_proj, temperature):
    x = np.asarray(x, dtype=np.float32)
    w_qkv = np.asarray(w_qkv, dtype=np.float32)
    w_dw = np.asarray(w_dw, dtype=np.float32)
    w_proj = np.asarray(w_proj, dtype=np.float32)
    temperature = np.asarray(temperature, dtype=np.float32)
    bf16 = mybir.dt.np(BF16)
    in_maps = []
    for c in range(N_CORES):
        n, g = c // 2, c % 2
        rows = np.concatenate(
            [256 * g + np.arange(256) + off for off in (0, 512, 1024)]
        )
        temps = temperature[0, HPC * g:HPC * g + HPC, 0, 0]
        in_maps.append(
            {
                "x": np.ascontiguousarray(x[n]).astype(bf16),
                "wqkvT": np.ascontiguousarray(w_qkv[rows, :, 0].T).astype(bf16),
                "wdw": np.ascontiguousarray(w_dw[rows, 0, :]),
                "wpT": np.ascontiguousarray(
                    w_proj[:, 256 * g:256 * g + 256, 0].T.reshape(HPC, DH, D)
                ).astype(bf16),
                "lntemp": _lntemp(temps),
                "ident": np.vstack([np.eye(DH, dtype=np.float32)] * 2).astype(bf16),
                "onesv": np.ones((P, NLC, 1), dtype=np.float32).astype(bf16),
                "onesc": np.ones((P, 1), dtype=np.float32).astype(bf16),
            }
        )
    return in_maps


_PROGRAM = None


def _get_program():
    global _PROGRAM
    if _PROGRAM is None:
        _PROGRAM = build_program()
    return _PROGRAM


def kernel(x, w_qkv, w_dw, w_proj, temperature):
    prog = _get_program()
    in_maps = make_in_maps(x, w_qkv, w_dw, w_proj, temperature)
    res = run_bass_kernel_spmd(prog, in_maps, list(range(N_CORES)))
    y = np.empty((N, D, L), np.float32)
    for n in range(N):
        y[n] = res.results[2 * n]["y"] + res.results[2 * n + 1]["y"]
    return y


if __name__ == "__main__":
    prog = build_program()
    print("program built ok")


# revision 58
# speedup vs baseline: 8.3027x; 8.3027x over previous
"""Trainium2 Bass kernel for the MGA dense-transformer block (v3).

Reference computation (per batch n):
    qkv = depthwise3(conv1x1(x, w_qkv), w_dw)         # (3D, L)
    q,k,v per head (dh=64), l2-normalized q,k, scores = q k^T * temp,
    softmax over keys, out = attn @ v, y = conv1x1(out, w_proj)

Sharding over 8 cores: core c -> (batch n = c//2, head group g = c%2 of 4
heads).  Each core computes its 768 qkv channels, runs attention for its 4
heads, and produces a partial projection y_partial = Wp[:, cols_g] @ out_g
(512, 2048).  Host sums the two partials per batch.

Engine balance (v3):
  * everything bf16 except PSUM accumulation and the norm/denominator
    chains: halves DMA + SBUF and doubles DVE throughput on the
    depthwise taps; matmuls are 1 cycle/row either way.
  * depthwise: center tap on Pool, side taps on DVE (2x bf16 mode),
    PSUM drain via Act in the prologue chunks / DVE in the steady state.
  * l2 norms: Pool squares + partition_all_reduce (result lands on all
    64 partitions), then rsqrt = exp(-0.5*ln(s)) on Act -- ln and exp
    share one activation table so the Act engine never reloads tables;
    temperature enters as an exp bias (host sends ln(temp)).
  * attention: scores f32 PSUM -> Act exp -> bf16 stripes -> PV with a
    ones row appended to v^T for the softmax denominator; 1/denom via
    DVE reciprocal + Pool partition_broadcast.
  * conv chunks for heads 2,3 are emitted between head-0 attention
    lt-blocks; projection runs per-lt right behind head-3 attention.
"""

from contextlib import ExitStack

import numpy as np

import concourse.bacc as bacc
import concourse.bass_isa as bass_isa
import concourse.mybir as mybir
import concourse.tile as tile
from concourse.bass_utils import run_bass_kernel_spmd

F32 = mybir.dt.float32
F32R = mybir.dt.float32r
BF16 = mybir.dt.bfloat16
AF = mybir.ActivationFunctionType
RED = bass_isa.ReduceOp

N, D, L, H = 4, 512, 2048, 8
DH = D // H          # 64 head dim
HPC = H // 2         # 4 heads per core
C = 3 * 256          # 768 shard qkv channels
P = 128
NLT = L // 512       # 4 query tiles
NLC = L // 128       # 16 key chunks
N_CORES = 8


def build_program(debug_dumps=False, repeat=1):
    nc = bacc.Bacc("TRN2", target_bir_lowering=False, debug=False)
    dbg = {}
    if debug_dumps:
        dbg["pre0"] = nc.dram_tensor("dbg_pre0", (P, L), BF16, kind="ExternalOutput")
        dbg["dw0"] = nc.dram_tensor("dbg_dw0", (P, L), BF16, kind="ExternalOutput")
        dbg["sq0"] = nc.dram_tensor("dbg_sq0", (P, L), BF16, kind="ExternalOutput")
        dbg["ss0"] = nc.dram_tensor("dbg_ss0", (P, L), F32, kind="ExternalOutput")
        dbg["qn0"] = nc.dram_tensor("dbg_qn0", (DH, L), BF16, kind="ExternalOutput")
        dbg["kn0"] = nc.dram_tensor("dbg_kn0", (DH, L), BF16, kind="ExternalOutput")
        dbg["vt0"] = nc.dram_tensor(
            "dbg_vt0", (P, NLC, DH + 1), BF16, kind="ExternalOutput"
        )
        dbg["es0"] = nc.dram_tensor("dbg_es0", (P, 4, 512), BF16, kind="ExternalOutput")
        dbg["outn0"] = nc.dram_tensor("dbg_outn0", (DH, L), BF16, kind="ExternalOutput")

    x_d = nc.dram_tensor("x", (D, L), BF16, kind="ExternalInput")
    wqkvT_d = nc.dram_tensor("wqkvT", (D, C), BF16, kind="ExternalInput")
    wdw_d = nc.dram_tensor("wdw", (C, 3), F32, kind="ExternalInput")
    wpT_d = nc.dram_tensor("wpT", (HPC, DH, D), BF16, kind="ExternalInput")
    # col hl: row 0 = ln(temp_hl) (q), row 32 = 0 (k); bias of the rsqrt exp
    lntemp_d = nc.dram_tensor("lntemp", (P, HPC), F32, kind="ExternalInput")
    onesc_d = nc.dram_tensor("onesc", (P, 1), BF16, kind="ExternalInput")
    onesr_d = nc.dram_tensor("onesr", (P, P), F32R, kind="ExternalInput")
    # two stacked copies of eye(64) so a slice exists at base partition 0 and 64
    ident_d = nc.dram_tensor("ident", (2 * DH, DH), BF16, kind="ExternalInput")
    onesv_d = nc.dram_tensor("onesv", (P, NLC, 1), BF16, kind="ExternalInput")
    y_d = nc.dram_tensor("y", (D, L), F32, kind="ExternalOutput")

    with tile.TileContext(nc) as tc, ExitStack() as ctx:
        wp = ctx.enter_context(tc.tile_pool(name="w", bufs=1))
        xp = ctx.enter_context(tc.tile_pool(name="xp", bufs=4))
        onp = ctx.enter_context(tc.tile_pool(name="onp", bufs=8))
        esp = ctx.enter_context(tc.tile_pool(name="esp", bufs=4))
        prep = ctx.enter_context(tc.tile_pool(name="prep", bufs=2))
        dwp = ctx.enter_context(tc.tile_pool(name="dwp", bufs=6))
        nqp = ctx.enter_context(tc.tile_pool(name="nqp", bufs=4))
        vtp = ctx.enter_context(tc.tile_pool(name="vtp", bufs=1))
        smp = ctx.enter_context(tc.tile_pool(name="smp", bufs=1))
        # PSUM: pa = 1-bank slots (conv pre, po, proj), 4 bufs;
        #       pb = 2-bank slots (score pairs, transposes), 2 bufs. 4+4=8.
        pap = ctx.enter_context(tc.tile_pool(name="pa", bufs=4, space="PSUM"))
        pbp = ctx.enter_context(tc.tile_pool(name="pb", bufs=2, space="PSUM"))

        # ---- weights / constants -------------------------------------------
        wq_sb = []
        for kc in range(4):
            t = wp.tile([P, C], BF16, tag=f"wq{kc}")
            nc.gpsimd.dma_start(t[:], wqkvT_d[kc * 128:(kc + 1) * 128, :])
            wq_sb.append(t)
        wdw_sb = []
        for cc in range(6):
            t = wp.tile([P, 3], F32, tag=f"wdw{cc}")
            nc.gpsimd.dma_start(t[:], wdw_d[cc * 128:(cc + 1) * 128, :])
            wdw_sb.append(t)
        wp_sb = []
        for hl in range(HPC):
            t = wp.tile([DH, D], BF16, tag=f"wp{hl}")
            nc.gpsimd.dma_start(t[:], wpT_d[hl, :, :])
            wp_sb.append(t)
        lntemp_sb = wp.tile([P, HPC], F32, tag="lntemp")
        nc.gpsimd.dma_start(lntemp_sb[:], lntemp_d[:])
        ones_sb = wp.tile([P, 1], BF16, tag="onesc")
        nc.gpsimd.dma_start(ones_sb[:], onesc_d[:])
        onesr_sb = wp.tile([P, P], F32R, tag="onesr")
        nc.gpsimd.dma_start(onesr_sb[:], onesr_d[:])
        ident_sb = wp.tile([2 * DH, DH], BF16, tag="ident")
        nc.gpsimd.dma_start(ident_sb[:], ident_d[:])
        onesv_sb = wp.tile([P, NLC, 1], BF16, tag="onesv")
        nc.gpsimd.dma_start(onesv_sb[:], onesv_d[:])

        x_sb = []
        dw_sb = {}
        vt_sb = {}
        outn_sb = []

        def load_x(rep):
            x_sb.clear()
            for kc in range(4):
                t = xp.tile([P, L], BF16, tag="x", name=f"x{rep}_{kc}")
                nc.sync.dma_start(t[:], x_d[kc * 128:(kc + 1) * 128, :])
                x_sb.append(t)
            outn_sb.clear()
            outn_sb.extend(
                onp.tile([DH, L], BF16, tag="outn", name=f"outn{rep}_{i}")
                for i in range(HPC)
            )

        def conv_chunk(cc, drain_act=False):
            # drain_act: early chunks drain PSUM via Act (idle in prologue);
            # later chunks drain via DVE so Act's exp stream is not stalled.
            pre = prep.tile([P, L], BF16, tag="pre", name=f"pre{cc}")
            for lt in range(NLT):
                ps = pap.tile([P, 512], F32, tag="pa", name=f"cps{cc}_{lt}")
                for kc in range(4):
                    nc.tensor.matmul(
                        ps[:],
                        wq_sb[kc][:, cc * 128:(cc + 1) * 128],
                        x_sb[kc][:, lt * 512:(lt + 1) * 512],
                        start=(kc == 0),
                        stop=(kc == 3),
                    )
                if drain_act:
                    nc.scalar.copy(pre[:, lt * 512:(lt + 1) * 512], ps[:])
                else:
                    nc.vector.tensor_copy(pre[:, lt * 512:(lt + 1) * 512], ps[:])
            dw = dwp.tile([P, L], BF16, tag="dw", name=f"dw{cc}")
            nc.vector.tensor_scalar_mul(dw[:], pre[:], wdw_sb[cc][:, 1:2])
            nc.vector.affine_then_add(
                dw[:, 1:L], pre[:, 0:L - 1], dw[:, 1:L],
                scale=wdw_sb[cc][:, 0:1], bias=0.0,
            )
            nc.vector.affine_then_add(
                dw[:, 0:L - 1], pre[:, 1:L], dw[:, 0:L - 1],
                scale=wdw_sb[cc][:, 2:3], bias=0.0,
            )
            dw_sb[cc] = dw
            if debug_dumps and cc == 0:
                nc.sync.dma_start(dbg["pre0"][:], pre[:])
                nc.sync.dma_start(dbg["dw0"][:], dw[:])

        def ch_slice(base, hl):
            c0 = base + DH * hl
            return dw_sb[c0 // 128][c0 % 128:c0 % 128 + DH, :]

        def normalize(hl):
            # l2-normalize q and k of head hl in place.  Sum-of-squares via
            # a ones-vector matmul into PSUM rows {0 (q), 32 (k)}; rsqrt as
            # exp(-0.5*ln(s) + ln(temp)) on Act -- ln/exp share one
            # activation table with the attention exp, so no table reloads.
            q, k = ch_slice(0, hl), ch_slice(256, hl)
            b = DH * (hl % 2)  # base partition of this head's q/k slices
            sqq = nqp.tile([P, L], BF16, tag="nq", name=f"sqq{hl}")
            sqk = nqp.tile([P, L], BF16, tag="nq", name=f"sqk{hl}")
            nc.vector.tensor_mul(sqq[b:b + DH, :], q[:], q[:])
            nc.vector.tensor_mul(sqk[b:b + DH, :], k[:], k[:])
            nrm = smp.tile([P, L], F32, tag="nrm", bufs=2, name=f"nrm{hl}")
            nrb = smp.tile([P, L], F32R, tag="nrb", bufs=2, name=f"nrb{hl}")
            for lt in range(NLT):
                ps = pap.tile([P, 512], F32, tag="pa", name=f"nps{hl}_{lt}")
                nc.tensor.matmul(
                    ps[0:1, :], ones_sb[b:b + DH, :],
                    sqq[b:b + DH, lt * 512:(lt + 1) * 512],
                    start=True, stop=True, skip_group_check=True,
                )
                nc.tensor.matmul(
                    ps[32:33, :], ones_sb[b:b + DH, :],
                    sqk[b:b + DH, lt * 512:(lt + 1) * 512],
                    start=True, stop=True, skip_group_check=True,
                )
                nc.scalar.activation(
                    nrm[0:33, lt * 512:(lt + 1) * 512], ps[0:33, :], AF.Ln
                )
            nc.scalar.activation(
                nrb[0:33, :], nrm[0:33, :], AF.Exp, scale=-0.5,
                bias=lntemp_sb[0:33, hl:hl + 1],
            )
            # broadcast temp/|q| and 1/|k| across the head's 64 partitions
            # with K=1 PE matmuls (ones column x rsqrt row), then scale q/k
            # straight from PSUM on DVE.
            for r, s in ((0, q), (32, k)):
                for lt in range(NLT):
                    bc = pap.tile([P, 512], F32, tag="pa", name=f"bc{hl}_{r}_{lt}")
                    nc.tensor.matmul(
                        bc[:], onesr_sb[r:r + 1, :],
                        nrb[r:r + 1, lt * 512:(lt + 1) * 512],
                        start=True, stop=True,
                    )
                    nc.vector.tensor_mul(
                        s[:, lt * 512:(lt + 1) * 512],
                        s[:, lt * 512:(lt + 1) * 512],
                        bc[b:b + DH, :],
                    )
            if debug_dumps and hl == 0:
                nc.sync.dma_start(dbg["sq0"][:], sqq[:])
                nc.sync.dma_start(dbg["ss0"][:], nrm[:])
                nc.sync.dma_start(dbg["qn0"][:], q[:])
                nc.sync.dma_start(dbg["kn0"][:], k[:])

        def build_vt(hl):
            # v^T plus a ones row for the softmax denominator, stored bf16
            v = ch_slice(512, hl)
            vt = vtp.tile([P, NLC, DH + 1], BF16, tag=f"vt{hl}", name=f"vt{hl}")
            nc.vector.tensor_copy(vt[:, :, DH:DH + 1], onesv_sb[:])
            vbase = (512 + DH * hl) % 128  # base partition of the v slice (0 or 64)
            ident = ident_sb[vbase:vbase + DH, :]
            # 4 transposes form one accumulation group in a single PSUM bank
            for lg in range(NLC // 4):
                ps = pbp.tile([P, 4, DH], BF16, tag="pb", name=f"tps{hl}_{lg}")
                for j in range(4):
                    lc = 4 * lg + j
                    nc.tensor.matmul(
                        ps[:, j, :], v[:, lc * 128:(lc + 1) * 128], ident,
                        is_transpose=True, start=(j == 0), stop=(j == 3),
                    )
                nc.vector.tensor_copy(vt[:, 4 * lg:4 * lg + 4, 0:DH], ps[:])
            vt_sb[hl] = vt
            if debug_dumps and hl == 0:
                nc.sync.dma_start(dbg["vt0"][:], vt[:])

        def attention_lt(hl, lt):
            q = ch_slice(0, hl)
            k = ch_slice(256, hl)
            vt = vt_sb[hl]
            stripes = [
                esp.tile([P, 4, 512], BF16, tag="es", name=f"es_{hl}_{lt}_{i}")
                for i in range(4)
            ]
            po = pap.tile([P, 512], F32, tag="pa", name=f"po{hl}_{lt}")
            for g in range(NLC // 2):
                ps4 = pbp.tile([P, 2, 512], F32, tag="pb", name=f"s{hl}_{lt}_{g}")
                for j in range(2):
                    lc = 2 * g + j
                    nc.tensor.matmul(
                        ps4[:, j, :],
                        k[:, lc * 128:(lc + 1) * 128],
                        q[:, lt * 512:(lt + 1) * 512],
                        start=True,
                        stop=True,
                    )
                st = stripes[g // 2]
                nc.scalar.activation(
                    st[:, 2 * (g % 2):2 * (g % 2) + 2, :], ps4[:], AF.Exp
                )
                for j in range(2):
                    lc = 2 * g + j
                    nc.tensor.matmul(
                        po[0:DH + 1, :],
                        vt[:, lc, :],
                        st[:, lc % 4, :],
                        start=(lc == 0),
                        stop=(lc == NLC - 1),
                    )
            if debug_dumps and hl == 0 and lt == 0:
                nc.sync.dma_start(dbg["es0"][:], stripes[0][:])
            rec = smp.tile([P, 512], F32R, tag="rec", bufs=2)
            with nc.allow_low_precision(reason="1/denom feeds an f32r matmul"):
                nc.vector.reciprocal(rec[DH:DH + 1, :], po[DH:DH + 1, :])
            # broadcast 1/denom across 64 partitions via a K=1 matmul, then
            # apply it to the PV numerators straight from PSUM.
            bcb = pbp.tile([DH, 512], F32, tag="pb", name=f"bcb{hl}_{lt}")
            nc.tensor.matmul(
                bcb[:], onesr_sb[DH:DH + 1, 0:DH], rec[DH:DH + 1, :],
                start=True, stop=True,
            )
            bcd = smp.tile([P, 512], F32, tag="bcd", bufs=2, name=f"bcd{hl}_{lt}")
            nc.vector.tensor_copy(bcd[0:DH, :], bcb[:])
            dst = outn_sb[hl][:, lt * 512:(lt + 1) * 512]
            nc.vector.tensor_mul(dst, po[0:DH, :], bcd[0:DH, :])
            if debug_dumps and hl == 0 and lt == NLT - 1:
                nc.sync.dma_start(dbg["outn0"][:], outn_sb[0][:])

        def projection_lt(rep, lt):
            for oc in range(4):
                ps = pap.tile([P, 512], F32, tag="pa", name=f"yps{rep}_{oc}{lt}")
                for hl in range(HPC):
                    nc.tensor.matmul(
                        ps[:],
                        wp_sb[hl][:, oc * 128:(oc + 1) * 128],
                        outn_sb[hl][:, lt * 512:(lt + 1) * 512],
                        start=(hl == 0),
                        stop=(hl == HPC - 1),
                    )
                ysb = smp.tile(
                    [P, 512], F32, tag="ysb", bufs=2, name=f"ysb{rep}_{oc}_{lt}"
                )
                nc.vector.tensor_copy(ysb[:], ps[:])
                nc.sync.dma_start(
                    y_d[oc * 128:(oc + 1) * 128, lt * 512:(lt + 1) * 512], ysb[:]
                )

        for rep in range(repeat):
            load_x(rep)
            # heads 0,1 inputs first so their attention can start while the
            # remaining conv chunks run between attention lt-blocks.
            conv_chunk(0, drain_act=True)
            conv_chunk(2, drain_act=True)
            normalize(0)
            normalize(1)
            conv_chunk(4, drain_act=True)
            build_vt(0)
            build_vt(1)
            attention_lt(0, 0)
            conv_chunk(1)
            attention_lt(0, 1)
            conv_chunk(3)
            attention_lt(0, 2)
            conv_chunk(5)
            attention_lt(0, 3)
            normalize(2)
            normalize(3)
            build_vt(2)
            build_vt(3)
            for lt in range(NLT):
                attention_lt(1, lt)
            for lt in range(NLT):
                attention_lt(2, lt)
                attention_lt(3, lt)
                projection_lt(rep, lt)

    nc.compile()
    return nc


def _lntemp(temps):
    # col hl: row 0 = ln(temp_hl) for q, row 32 = 0 for k, rest unused
    t = np.zeros((P, HPC), np.float32)
    t[0, :] = np.log(temps)
    return t


def make_in_maps(x, w_qkv, w_dw, w_proj, temperature):
    x = np.asarray(x, dtype=np.float32)
    w_qkv = np.asarray(w_qkv, dtype=np.float32)
    w_dw = np.asarray(w_dw, dtype=np.float32)
    w_proj = np.asarray(w_proj, dtype=np.float32)
    temperature = np.asarray(temperature, dtype=np.float32)
    bf16 = mybir.dt.np(BF16)
    in_maps = []
    for c in range(N_CORES):
        n, g = c // 2, c % 2
        rows = np.concatenate(
            [256 * g + np.arange(256) + off for off in (0, 512, 1024)]
        )
        temps = temperature[0, HPC * g:HPC * g + HPC, 0, 0]
        in_maps.append(
            {
                "x": np.ascontiguousarray(x[n]).astype(bf16),
                "wqkvT": np.ascontiguousarray(w_qkv[rows, :, 0].T).astype(bf16),
                "wdw": np.ascontiguousarray(w_dw[rows, 0, :]),
                "wpT": np.ascontiguousarray(
                    w_proj[:, 256 * g:256 * g + 256, 0].T.reshape(HPC, DH, D)
                ).astype(bf16),
                "lntemp": _lntemp(temps),
                "ident": np.vstack([np.eye(DH, dtype=np.float32)] * 2).astype(bf16),
                "onesv": np.ones((P, NLC, 1), dtype=np.float32).astype(bf16),
                "onesc": np.ones((P, 1), dtype=np.float32).astype(bf16),
                "onesr": np.ones((P, P), dtype=np.float32),
            }
        )
    return in_maps


_PROGRAM = None


def _get_program():
    global _PROGRAM
    if _PROGRAM is None:
        _PROGRAM = build_program()
    return _PROGRAM


def kernel(x, w_qkv, w_dw, w_proj, temperature):
    prog = _get_program()
    in_maps = make_in_maps(x, w_qkv, w_dw, w_proj, temperature)
    res = run_bass_kernel_spmd(prog, in_maps, list(range(N_CORES)))
    y = np.empty((N, D, L), np.float32)
    for n in range(N):
        y[n] = res.results[2 * n]["y"] + res.results[2 * n + 1]["y"]
    return y


if __name__ == "__main__":
    prog = build_program()
    print("program built ok")
